# revision 14
# baseline (speedup 1.0000x reference)
"""Trainium2 Bass kernel for nn_GCNLayer_36962488549946.

Per batch element b and point n: knn (K=8, incl. self) by pairwise squared
distance, gather neighbor features, feat = [neigh - x, x] @ W^T, BatchNorm
(inference), LeakyReLU(0.2), max over the 8 neighbors.

Algebraic refactor (exact: bn scale > 0, LeakyReLU monotone):
    out[n,u] = LRelu( max_k A[idx[n,k], u]  +  Bv[n, u] )
    A  = x @ (inv*W1)^T                 W1 = W[:, :C], inv = gamma*rsqrt(var+eps)
    Bv = x @ (inv*(W2-W1))^T + shift    W2 = W[:, C:], shift = beta - mean*inv
Ranking by score[n,m] = <x_n, x_m> - ||x_m||^2/2 (same per-row top-k order).

Distance scores use the exact-grade fp16 hi/lo split (x = h + l/32):
    score = [h_r; 1; 1]^T [h_m; sq_h; sq_l]            (K=66)
          + [h_r/32 ; 32*l_r]^T [32*l_m ; h_m/32]      (K=128)
A/Bv GEMMs use single-term fp16 (rel err ~2e-4, well within tolerance).
Top-8 via MAX8 + FIND_INDEX8 on the fp32 distance row (exact).

Main loop is software-pipelined: the post-gather tail (A-max reduce, Bv add,
LeakyReLU, store) of tile t is emitted after max8/find of tile t+1 so the
vector engine never stalls on the gather DMAs.

Sharding: 8 cores; core c handles batch b = c // 2, row half (c % 2).
"""

import numpy as np

import concourse.bacc as bacc
import concourse.bass as bass
import concourse.mybir as mybir
import concourse.tile as tile
from concourse.bass import ts
from concourse.bass_utils import run_bass_kernel_spmd
from concourse.masks import make_identity

F32 = mybir.dt.float32
F16 = mybir.dt.float16
U32 = mybir.dt.uint32
AF = mybir.ActivationFunctionType
ALU = mybir.AluOpType
AX = mybir.AxisListType

K = 8
BN_EPS = 1e-3
NEG_SLOPE = 0.2
SC = 32.0

B_FULL, N_FULL, C_FULL, U_FULL = 4, 8192, 64, 64
N_CORES = 8


def build_nc(N=N_FULL, ROWS=N_FULL // 2, C=C_FULL, U=U_FULL):
    assert N % 4096 == 0 and ROWS % 128 == 0 and C == 64 and U == 64
    n_mt = N // 128
    n_rt = ROWS // 128
    n_dk = N // 1024  # double-chunks per row tile
    n_q = N // 4096   # square-accum quarters

    nc = bacc.Bacc(trn_type="TRN2")

    xin = nc.declare_dram_parameter("xin", [N, C], F32, isOutput=False)
    xrows = nc.declare_dram_parameter("xrows", [ROWS, C], F32, isOutput=False)
    Wp = nc.declare_dram_parameter("W", [U, 2 * C], F32, isOutput=False)
    bn_gamma = nc.declare_dram_parameter("bn_gamma", [U], F32, isOutput=False)
    bn_beta = nc.declare_dram_parameter("bn_beta", [U], F32, isOutput=False)
    bn_mean = nc.declare_dram_parameter("bn_mean", [U], F32, isOutput=False)
    bn_var = nc.declare_dram_parameter("bn_var", [U], F32, isOutput=False)
    out = nc.declare_dram_parameter("out", [ROWS, U], F32, isOutput=True)

    A_dram = nc.dram_tensor("A_dram", [N, U], F16)

    with tile.TileContext(nc) as tc:
        with (
            tc.tile_pool(name="keep", bufs=1) as keep,
            tc.tile_pool(name="psS", bufs=2, space="PSUM") as psS,
        ):
            id128 = keep.tile([128, 128], F32)
            make_identity(nc, id128[:])
            id64 = keep.tile([64, 64], F32)
            make_identity(nc, id64[:])

            H66 = keep.tile([66, N], F16)      # [h_m ; sq_h ; sq_l]
            CR_m = keep.tile([128, N], F16)    # [32*l_m ; h_m/32]
            Hr66 = keep.tile([66, ROWS], F16)  # [h_r ; 1 ; 1]
            CR_r = keep.tile([128, ROWS], F16)  # [h_r/32 ; 32*l_r]
            ra_h = keep.tile([64, U], F16)      # (inv*W1)^T
            rw_h = keep.tile([65, U], F16)      # [(inv*(W2-W1))^T ; shift]

            # ---------- BN inv (partition = u) ----------
            g64 = keep.tile([64, 1], F32)
            nc.sync.dma_start(out=g64[:], in_=bn_gamma[:, None])
            v64 = keep.tile([64, 1], F32)
            nc.sync.dma_start(out=v64[:], in_=bn_var[:, None])
            inv64 = keep.tile([64, 1], F32)
            nc.vector.tensor_scalar(
                out=inv64[:], in0=v64[:], scalar1=BN_EPS, scalar2=None, op0=ALU.add
            )
            nc.scalar.sqrt(out=inv64[:], in_=inv64[:])
            nc.vector.reciprocal(out=inv64[:], in_=inv64[:])
            nc.vector.tensor_mul(out=inv64[:], in0=inv64[:], in1=g64[:])

            # ---------- shift row (partition 64 of rw_h) ----------
            b1 = keep.tile([65, U], F32)
            nc.sync.dma_start(out=b1[64:65, :], in_=bn_beta[None, :])
            m1 = keep.tile([65, U], F32)
            nc.sync.dma_start(out=m1[64:65, :], in_=bn_mean[None, :])
            g1 = keep.tile([65, U], F32)
            nc.sync.dma_start(out=g1[64:65, :], in_=bn_gamma[None, :])
            v1 = keep.tile([65, U], F32)
            nc.sync.dma_start(out=v1[64:65, :], in_=bn_var[None, :])
            t1 = keep.tile([65, U], F32)
            nc.vector.tensor_scalar(
                out=t1[64:65, :], in0=v1[64:65, :], scalar1=BN_EPS, scalar2=None,
                op0=ALU.add,
            )
            nc.scalar.sqrt(out=t1[64:65, :], in_=t1[64:65, :])
            nc.vector.reciprocal(out=t1[64:65, :], in_=t1[64:65, :])
            nc.vector.tensor_mul(out=t1[64:65, :], in0=t1[64:65, :], in1=g1[64:65, :])
            nc.vector.tensor_mul(out=m1[64:65, :], in0=m1[64:65, :], in1=t1[64:65, :])
            nc.vector.tensor_sub(out=t1[64:65, :], in0=b1[64:65, :], in1=m1[64:65, :])
            nc.scalar.copy(out=rw_h[64:65, :], in_=t1[64:65, :])

            # ---------- scaled weights, transposed ----------
            W_sb = keep.tile([64, 2 * C], F32)
            nc.sync.dma_start(out=W_sb[:], in_=Wp[:, :])
            Wsc = keep.tile([64, 2 * C], F32)
            nc.vector.tensor_scalar(
                out=Wsc[:], in0=W_sb[:], scalar1=inv64[:, 0:1], scalar2=None,
                op0=ALU.mult,
            )
            wt_ps = psS.tile([2 * C, U], F32, tag="small")
            nc.tensor.transpose(out=wt_ps[:], in_=Wsc[:], identity=id64[:])
            ra32 = keep.tile([64, U], F32)
            nc.vector.tensor_copy(out=ra32[:], in_=wt_ps[0:C, :])
            nc.scalar.copy(out=ra_h[:], in_=ra32[:])
            rwdiff = keep.tile([64, U], F32)
            nc.vector.tensor_sub(out=rwdiff[:], in0=wt_ps[C:2 * C, :], in1=ra32[:])
            nc.scalar.copy(out=rw_h[0:64, :], in_=rwdiff[:])

            # ---------- preprocessing ----------
            # squares first: -||x||^2/2 as fp16 hi/lo rows of H66 (independent)
            with tc.tile_pool(name="sqp", bufs=2) as sqp:
                for i in range(n_q):
                    qv = xin[i * 4096:(i + 1) * 4096, :].rearrange(
                        "(p t) c -> p (t c)", p=128
                    )
                    qt = sqp.tile([128, 2048], F32, tag="sqload")
                    nc.sync.dma_start(out=qt[:], in_=qv)
                    nc.scalar.square(out=qt[:], in_=qt[:])
                    sqq = sqp.tile([128, 32], F32, tag="sqred")
                    nc.vector.tensor_reduce(
                        out=sqq[:], in_=qt[:].rearrange("p (t c) -> p t c", c=64),
                        axis=AX.X, op=ALU.add,
                    )
                    nc.vector.tensor_scalar(
                        out=sqq[:], in0=sqq[:], scalar1=-0.5, scalar2=None,
                        op0=ALU.mult,
                    )
                    sq_h = sqp.tile([128, 32], F16, tag="sqh")
                    nc.scalar.copy(out=sq_h[:], in_=sqq[:])
                    sqr = sqp.tile([128, 32], F32, tag="sqr")
                    nc.vector.tensor_sub(out=sqr[:], in0=sqq[:], in1=sq_h[:])
                    sq_l = sqp.tile([128, 32], F16, tag="sql")
                    nc.scalar.copy(out=sq_l[:], in_=sqr[:])
                    nc.sync.dma_start(
                        out=H66[64:65, i * 4096:(i + 1) * 4096], in_=sq_h[:]
                    )
                    nc.sync.dma_start(
                        out=H66[65:66, i * 4096:(i + 1) * 4096], in_=sq_l[:]
                    )

            with tc.tile_pool(name="scr", bufs=1) as scr, \
                 tc.tile_pool(name="half", bufs=2) as halfp, \
                 tc.tile_pool(name="ld", bufs=3) as ld:
                xT32 = scr.tile([64, N], F32)

                def trans_block(dst, src_dram, i):
                    """Transpose 4 consecutive 128-row tiles into dst[:, i*512:]."""
                    tp = psS.tile([64, 512], F32, tag="small")
                    xt4 = ld.tile([128, 4, C], F32, tag="xload")
                    nc.sync.dma_start(
                        out=xt4[:],
                        in_=src_dram[ts(i, 512), :].rearrange(
                            "(j p) c -> p j c", p=128
                        ),
                    )
                    for j in range(4):
                        nc.tensor.transpose(
                            out=tp[:, ts(j, 128)], in_=xt4[:, j, :],
                            identity=id128[:],
                        )
                    if i % 2 == 0:
                        nc.scalar.copy(out=dst[:, ts(i, 512)], in_=tp[:])
                    else:
                        nc.vector.tensor_copy(out=dst[:, ts(i, 512)], in_=tp[:])

                qw = N // 4
                for z in range(4):
                    for i in range(4 * z, 4 * z + 4):
                        trans_block(xT32, xin, i)
                    sl = slice(z * qw, (z + 1) * qw)
                    # h (fp16) for this quarter
                    nc.vector.tensor_copy(out=H66[0:64, sl], in_=xT32[:, sl])
                    # A table for this quarter (2 groups of 8 tiles)
                    for g in range(2):
                        base = 16 * z + 8 * g
                        pa = psS.tile([128, 512], F32, tag="small")
                        for j in range(8):
                            nc.tensor.matmul(
                                out=pa[:, ts(j, 64)],
                                lhsT=H66[0:64, ts(base + j, 128)],
                                rhs=ra_h[:, :], start=True, stop=True,
                            )
                        asb = ld.tile([128, 512], F16, tag="aev")
                        nc.scalar.copy(out=asb[:], in_=pa[:])
                        nc.sync.dma_start(
                            out=A_dram[base * 128:(base + 8) * 128, :].rearrange(
                                "(j p) c -> p j c", p=128
                            ),
                            in_=asb[:].rearrange("p (j c) -> p j c", c=64),
                        )
                    # residual + CR rows
                    res = halfp.tile([64, qw], F32, tag="res")
                    nc.vector.tensor_sub(
                        out=res[:], in0=xT32[:, sl], in1=H66[0:64, sl]
                    )
                    nc.scalar.activation(
                        out=CR_m[0:64, sl], in_=res[:], func=AF.Copy, scale=SC
                    )
                    h32s = halfp.tile([64, qw], F16, tag="h32s")
                    nc.vector.tensor_scalar(
                        out=h32s[:], in0=xT32[:, sl], scalar1=1.0 / SC,
                        scalar2=None, op0=ALU.mult,
                    )
                    nc.sync.dma_start(out=CR_m[64:128, sl], in_=h32s[:])

                # xrows transposes + h/l/CR rows
                xrT32 = scr.tile([64, ROWS], F32)
                nc.vector.memset(Hr66[64:66, :], 1.0)
                hw = ROWS // 2
                for z in range(2):
                    for i in range(z * (n_rt // 8), (z + 1) * (n_rt // 8)):
                        trans_block(xrT32, xrows, i)
                    sl = slice(z * hw, (z + 1) * hw)
                    nc.vector.tensor_copy(out=Hr66[0:64, sl], in_=xrT32[:, sl])
                    resr = halfp.tile([64, hw], F32, tag="resr")
                    nc.vector.tensor_sub(
                        out=resr[:], in0=xrT32[:, sl], in1=Hr66[0:64, sl]
                    )
                    nc.scalar.activation(
                        out=CR_r[0:64, sl], in_=xrT32[:, sl], func=AF.Copy,
                        scale=1.0 / SC,
                    )
                    l32s = halfp.tile([64, hw], F16, tag="l32s")
                    nc.scalar.activation(
                        out=l32s[:], in_=resr[:], func=AF.Copy, scale=SC
                    )
                    nc.sync.dma_start(out=CR_r[64:128, sl], in_=l32s[:])

            # gathers read A_dram; Tile does not track DRAM deps
            tc.strict_bb_all_engine_barrier()

            # ---------- main loop (software-pipelined tail) ----------
            with (
                tc.tile_pool(name="distp", bufs=2) as distp,
                tc.tile_pool(name="work", bufs=3) as work,
                tc.tile_pool(name="psD", bufs=3, space="PSUM") as psD,
            ):
                def emit_tail(t, gath):
                    # A-max over the 8 gathered rows: pairwise folds on gpsimd
                    # (keeps the vector engine free for max8/find_index8)
                    f1 = work.tile([128, 4, U], F16, tag="f1")
                    nc.vector.tensor_tensor(
                        out=f1[:], in0=gath[:, 0:4, :], in1=gath[:, 4:8, :],
                        op=ALU.max,
                    )
                    f2 = work.tile([128, 2, U], F16, tag="f2")
                    nc.vector.tensor_tensor(
                        out=f2[:], in0=f1[:, 0:2, :], in1=f1[:, 2:4, :],
                        op=ALU.max,
                    )
                    pb = psS.tile([128, U], F32, tag="small")
                    nc.tensor.matmul(
                        out=pb[:], lhsT=Hr66[0:65, ts(t, 128)], rhs=rw_h[:, :],
                        start=True, stop=True,
                    )
                    pbs = work.tile([128, U], F32, tag="pbs")
                    nc.scalar.copy(out=pbs[:], in_=pb[:])
                    amax = work.tile([128, U], F32, tag="amax")
                    nc.vector.tensor_tensor(
                        out=amax[:], in0=f2[:, 0, :], in1=f2[:, 1, :], op=ALU.max,
                    )
                    acc = work.tile([128, U], F32, tag="acc")
                    nc.gpsimd.tensor_tensor(
                        out=acc[:], in0=amax[:], in1=pbs[:], op=ALU.add,
                    )
                    # LeakyReLU(0.2): x -> 0.6*x + 0.4*|x| (Pool has no max op)
                    t04 = work.tile([128, U], F32, tag="t04")
                    nc.scalar.activation(
                        out=t04[:], in_=acc[:], func=AF.Abs, scale=0.4
                    )
                    a06 = work.tile([128, U], F32, tag="a06")
                    nc.scalar.activation(
                        out=a06[:], in_=acc[:], func=AF.Copy, scale=0.6
                    )
                    ot = work.tile([128, U], F32, tag="ot")
                    nc.gpsimd.tensor_tensor(
                        out=ot[:], in0=a06[:], in1=t04[:], op=ALU.add,
                    )
                    nc.sync.dma_start(out=out[ts(t, 128), :], in_=ot[:])

                prev = None
                for t in range(n_rt):
                    dist = distp.tile([128, N], F32, tag="dist")
                    for j in range(n_dk):
                        pd = psD.tile([128, 1024], F32, tag="pd")
                        for h in range(2):
                            ck = 2 * j + h
                            nc.tensor.matmul(
                                out=pd[:, ts(h, 512)], lhsT=Hr66[:, ts(t, 128)],
                                rhs=H66[:, ts(ck, 512)], start=True, stop=False,
                            )
                            nc.tensor.matmul(
                                out=pd[:, ts(h, 512)], lhsT=CR_r[:, ts(t, 128)],
                                rhs=CR_m[:, ts(ck, 512)], start=False, stop=True,
                            )
                        nc.scalar.copy(out=dist[:, ts(j, 1024)], in_=pd[:])
                    vals = work.tile([128, K], F32, tag="vals")
                    nc.vector.max(out=vals[:], in_=dist[:])
                    idx = work.tile([128, K], U32, tag="idx")
                    nc.vector.max_index(out=idx[:], in_max=vals[:], in_values=dist[:])
                    gath = work.tile([128, K, U], F16, tag="gath")
                    for k2 in range(K):
                        nc.gpsimd.indirect_dma_start(
                            out=gath[:, k2, :], out_offset=None, in_=A_dram[:],
                            in_offset=bass.IndirectOffsetOnAxis(
                                ap=idx[:, k2:k2 + 1], axis=0
                            ),
                        )
                    if prev is not None:
                        emit_tail(*prev)
                    prev = (t, gath)
                emit_tail(*prev)

    nc.finalize()
    return nc


_NC_CACHE = {}


def _get_nc(N, ROWS, C, U):
    key = (N, ROWS, C, U)
    if key not in _NC_CACHE:
        _NC_CACHE[key] = build_nc(N=N, ROWS=ROWS, C=C, U=U)
    return _NC_CACHE[key]


def kernel(inputs, W, bn_gamma, bn_beta, bn_mean, bn_var, _trace=False):
    """Full-problem entry: takes unsharded inputs, returns (B, N, U) float32."""
    inputs = np.ascontiguousarray(np.asarray(inputs, dtype=np.float32))
    W = np.ascontiguousarray(np.asarray(W, dtype=np.float32))
    bn_gamma = np.asarray(bn_gamma, dtype=np.float32)
    bn_beta = np.asarray(bn_beta, dtype=np.float32)
    bn_mean = np.asarray(bn_mean, dtype=np.float32)
    bn_var = np.asarray(bn_var, dtype=np.float32)

    B, N, C = inputs.shape
    U = W.shape[0]
    assert B * 2 == N_CORES
    ROWS = N // 2

    nc = _get_nc(N, ROWS, C, U)

    in_maps = []
    for c in range(N_CORES):
        b, half = c // 2, c % 2
        in_maps.append({
            "xin": inputs[b],
            "xrows": inputs[b, half * ROWS:(half + 1) * ROWS],
            "W": W,
            "bn_gamma": bn_gamma,
            "bn_beta": bn_beta,
            "bn_mean": bn_mean,
            "bn_var": bn_var,
        })

    res = run_bass_kernel_spmd(nc, in_maps, list(range(N_CORES)), trace=_trace)

    outp = np.empty((B, N, U), dtype=np.float32)
    for c in range(N_CORES):
        b, half = c // 2, c % 2
        outp[b, half * ROWS:(half + 1) * ROWS] = res.results[c]["out"]
    if _trace:
        return outp, res
    return outp


# revision 15
# speedup vs baseline: 1.1793x; 1.1793x over previous
"""Trainium2 Bass kernel for nn_GCNLayer_36962488549946.

Per batch element b and point n: knn (K=8, incl. self) by pairwise squared
distance, gather neighbor features, feat = [neigh - x, x] @ W^T, BatchNorm
(inference), LeakyReLU(0.2), max over the 8 neighbors.

Algebraic refactor (exact: bn scale > 0, LeakyReLU monotone):
    out[n,u] = LRelu( max_k A[idx[n,k], u]  +  Bv[n, u] )
    A  = x @ (inv*W1)^T                 W1 = W[:, :C], inv = gamma*rsqrt(var+eps)
    Bv = x @ (inv*(W2-W1))^T + shift    W2 = W[:, C:], shift = beta - mean*inv
Ranking by score[n,m] = <x_n, x_m> - ||x_m||^2/2 (same per-row top-k order).

Distance scores use the exact-grade fp16 hi/lo split (x = h + l/32):
    score = [h_r; 1; 1]^T [h_m; sq_h; sq_l]            (K=66)
          + [h_r/32 ; 32*l_r]^T [32*l_m ; h_m/32]      (K=128)
A/Bv GEMMs use single-term fp16 (rel err ~2e-4, well within tolerance).
Top-8 via MAX8 + FIND_INDEX8 on the fp32 distance row (exact).

Main loop is software-pipelined: the post-gather tail (A-max reduce, Bv add,
LeakyReLU, store) of tile t is emitted after max8/find of tile t+1 so the
vector engine never stalls on the gather DMAs.

Sharding: 8 cores; core c handles batch b = c // 2, row half (c % 2).
"""

import numpy as np

import concourse.bacc as bacc
import concourse.bass as bass
import concourse.mybir as mybir
import concourse.tile as tile
from concourse.bass import ts
from concourse.bass_utils import run_bass_kernel_spmd
from concourse.masks import make_identity

F32 = mybir.dt.float32
F16 = mybir.dt.float16
U32 = mybir.dt.uint32
AF = mybir.ActivationFunctionType
ALU = mybir.AluOpType
AX = mybir.AxisListType

K = 8
BN_EPS = 1e-3
NEG_SLOPE = 0.2
SC = 32.0

B_FULL, N_FULL, C_FULL, U_FULL = 4, 8192, 64, 64
N_CORES = 8


def build_nc(N=N_FULL, ROWS=N_FULL // 2, C=C_FULL, U=U_FULL):
    assert N % 4096 == 0 and ROWS % 128 == 0 and C == 64 and U == 64
    n_mt = N // 128
    n_rt = ROWS // 128
    n_dk = N // 1024  # double-chunks per row tile
    n_q = N // 4096   # square-accum quarters

    nc = bacc.Bacc(trn_type="TRN2")

    xin = nc.declare_dram_parameter("xin", [N, C], F32, isOutput=False)
    xrows = nc.declare_dram_parameter("xrows", [ROWS, C], F32, isOutput=False)
    Wp = nc.declare_dram_parameter("W", [U, 2 * C], F32, isOutput=False)
    bn_gamma = nc.declare_dram_parameter("bn_gamma", [U], F32, isOutput=False)
    bn_beta = nc.declare_dram_parameter("bn_beta", [U], F32, isOutput=False)
    bn_mean = nc.declare_dram_parameter("bn_mean", [U], F32, isOutput=False)
    bn_var = nc.declare_dram_parameter("bn_var", [U], F32, isOutput=False)
    out = nc.declare_dram_parameter("out", [ROWS, U], F32, isOutput=True)

    A_dram = nc.dram_tensor("A_dram", [N, U], F16)

    with tile.TileContext(nc) as tc:
        with (
            tc.tile_pool(name="keep", bufs=1) as keep,
            tc.tile_pool(name="psS", bufs=2, space="PSUM") as psS,
        ):
            id128 = keep.tile([128, 128], F32)
            make_identity(nc, id128[:])
            id64 = keep.tile([64, 64], F32)
            make_identity(nc, id64[:])

            H66 = keep.tile([66, N], F16)      # [h_m ; sq_h ; sq_l]
            CR_m = keep.tile([128, N], F16)    # [32*l_m ; h_m/32]
            Hr66 = keep.tile([66, ROWS], F16)  # [h_r ; 1 ; 1]
            CR_r = keep.tile([128, ROWS], F16)  # [h_r/32 ; 32*l_r]
            ra_h = keep.tile([64, U], F16)      # (inv*W1)^T
            rw_h = keep.tile([65, U], F16)      # [(inv*(W2-W1))^T ; shift]

            # ---------- BN inv (partition = u) ----------
            g64 = keep.tile([64, 1], F32)
            nc.sync.dma_start(out=g64[:], in_=bn_gamma[:, None])
            v64 = keep.tile([64, 1], F32)
            nc.sync.dma_start(out=v64[:], in_=bn_var[:, None])
            inv64 = keep.tile([64, 1], F32)
            nc.vector.tensor_scalar(
                out=inv64[:], in0=v64[:], scalar1=BN_EPS, scalar2=None, op0=ALU.add
            )
            nc.scalar.sqrt(out=inv64[:], in_=inv64[:])
            nc.vector.reciprocal(out=inv64[:], in_=inv64[:])
            nc.vector.tensor_mul(out=inv64[:], in0=inv64[:], in1=g64[:])

            # ---------- shift row (partition 64 of rw_h) ----------
            b1 = keep.tile([65, U], F32)
            nc.sync.dma_start(out=b1[64:65, :], in_=bn_beta[None, :])
            m1 = keep.tile([65, U], F32)
            nc.sync.dma_start(out=m1[64:65, :], in_=bn_mean[None, :])
            g1 = keep.tile([65, U], F32)
            nc.sync.dma_start(out=g1[64:65, :], in_=bn_gamma[None, :])
            v1 = keep.tile([65, U], F32)
            nc.sync.dma_start(out=v1[64:65, :], in_=bn_var[None, :])
            t1 = keep.tile([65, U], F32)
            nc.vector.tensor_scalar(
                out=t1[64:65, :], in0=v1[64:65, :], scalar1=BN_EPS, scalar2=None,
                op0=ALU.add,
            )
            nc.scalar.sqrt(out=t1[64:65, :], in_=t1[64:65, :])
            nc.vector.reciprocal(out=t1[64:65, :], in_=t1[64:65, :])
            nc.vector.tensor_mul(out=t1[64:65, :], in0=t1[64:65, :], in1=g1[64:65, :])
            nc.vector.tensor_mul(out=m1[64:65, :], in0=m1[64:65, :], in1=t1[64:65, :])
            nc.vector.tensor_sub(out=t1[64:65, :], in0=b1[64:65, :], in1=m1[64:65, :])
            nc.scalar.copy(out=rw_h[64:65, :], in_=t1[64:65, :])

            # ---------- scaled weights, transposed ----------
            W_sb = keep.tile([64, 2 * C], F32)
            nc.sync.dma_start(out=W_sb[:], in_=Wp[:, :])
            Wsc = keep.tile([64, 2 * C], F32)
            nc.vector.tensor_scalar(
                out=Wsc[:], in0=W_sb[:], scalar1=inv64[:, 0:1], scalar2=None,
                op0=ALU.mult,
            )
            wt_ps = psS.tile([2 * C, U], F32, tag="small")
            nc.tensor.transpose(out=wt_ps[:], in_=Wsc[:], identity=id64[:])
            ra32 = keep.tile([64, U], F32)
            nc.vector.tensor_copy(out=ra32[:], in_=wt_ps[0:C, :])
            nc.scalar.copy(out=ra_h[:], in_=ra32[:])
            rwdiff = keep.tile([64, U], F32)
            nc.vector.tensor_sub(out=rwdiff[:], in0=wt_ps[C:2 * C, :], in1=ra32[:])
            nc.scalar.copy(out=rw_h[0:64, :], in_=rwdiff[:])

            # ---------- preprocessing ----------
            # squares first: -||x||^2/2 as fp16 hi/lo rows of H66 (independent)
            with tc.tile_pool(name="sqp", bufs=2) as sqp:
                for i in range(n_q):
                    qv = xin[i * 4096:(i + 1) * 4096, :].rearrange(
                        "(p t) c -> p (t c)", p=128
                    )
                    qt = sqp.tile([128, 2048], F32, tag="sqload")
                    nc.sync.dma_start(out=qt[:], in_=qv)
                    nc.scalar.square(out=qt[:], in_=qt[:])
                    sqq = sqp.tile([128, 32], F32, tag="sqred")
                    nc.vector.tensor_reduce(
                        out=sqq[:], in_=qt[:].rearrange("p (t c) -> p t c", c=64),
                        axis=AX.X, op=ALU.add,
                    )
                    nc.vector.tensor_scalar(
                        out=sqq[:], in0=sqq[:], scalar1=-0.5, scalar2=None,
                        op0=ALU.mult,
                    )
                    sq_h = sqp.tile([128, 32], F16, tag="sqh")
                    nc.scalar.copy(out=sq_h[:], in_=sqq[:])
                    sqr = sqp.tile([128, 32], F32, tag="sqr")
                    nc.vector.tensor_sub(out=sqr[:], in0=sqq[:], in1=sq_h[:])
                    sq_l = sqp.tile([128, 32], F16, tag="sql")
                    nc.scalar.copy(out=sq_l[:], in_=sqr[:])
                    nc.sync.dma_start(
                        out=H66[64:65, i * 4096:(i + 1) * 4096], in_=sq_h[:]
                    )
                    nc.sync.dma_start(
                        out=H66[65:66, i * 4096:(i + 1) * 4096], in_=sq_l[:]
                    )

            with tc.tile_pool(name="scr", bufs=1) as scr, \
                 tc.tile_pool(name="half", bufs=2) as halfp, \
                 tc.tile_pool(name="ld", bufs=3) as ld:
                xT32 = scr.tile([64, N], F32)

                def trans_block(dst, src_dram, i):
                    """Transpose 4 consecutive 128-row tiles into dst[:, i*512:]."""
                    tp = psS.tile([64, 512], F32, tag="small")
                    xt4 = ld.tile([128, 4, C], F32, tag="xload")
                    nc.sync.dma_start(
                        out=xt4[:],
                        in_=src_dram[ts(i, 512), :].rearrange(
                            "(j p) c -> p j c", p=128
                        ),
                    )
                    for j in range(4):
                        nc.tensor.transpose(
                            out=tp[:, ts(j, 128)], in_=xt4[:, j, :],
                            identity=id128[:],
                        )
                    if i % 2 == 0:
                        nc.scalar.copy(out=dst[:, ts(i, 512)], in_=tp[:])
                    else:
                        nc.vector.tensor_copy(out=dst[:, ts(i, 512)], in_=tp[:])

                qw = N // 4
                for z in range(4):
                    for i in range(4 * z, 4 * z + 4):
                        trans_block(xT32, xin, i)
                    sl = slice(z * qw, (z + 1) * qw)
                    # h (fp16) for this quarter
                    nc.vector.tensor_copy(out=H66[0:64, sl], in_=xT32[:, sl])
                    # A table for this quarter (2 groups of 8 tiles)
                    for g in range(2):
                        base = 16 * z + 8 * g
                        pa = psS.tile([128, 512], F32, tag="small")
                        for j in range(8):
                            nc.tensor.matmul(
                                out=pa[:, ts(j, 64)],
                                lhsT=H66[0:64, ts(base + j, 128)],
                                rhs=ra_h[:, :], start=True, stop=True,
                            )
                        asb = ld.tile([128, 512], F16, tag="aev")
                        nc.scalar.copy(out=asb[:], in_=pa[:])
                        nc.sync.dma_start(
                            out=A_dram[base * 128:(base + 8) * 128, :].rearrange(
                                "(j p) c -> p j c", p=128
                            ),
                            in_=asb[:].rearrange("p (j c) -> p j c", c=64),
                        )
                    # residual + CR rows
                    res = halfp.tile([64, qw], F32, tag="res")
                    nc.vector.tensor_sub(
                        out=res[:], in0=xT32[:, sl], in1=H66[0:64, sl]
                    )
                    if z % 2 == 0:
                        nc.scalar.activation(
                            out=CR_m[0:64, sl], in_=res[:], func=AF.Copy, scale=SC
                        )
                    else:
                        nc.vector.tensor_scalar(
                            out=CR_m[0:64, sl], in0=res[:], scalar1=SC,
                            scalar2=None, op0=ALU.mult,
                        )
                    h32s = halfp.tile([64, qw], F16, tag="h32s")
                    nc.vector.tensor_scalar(
                        out=h32s[:], in0=xT32[:, sl], scalar1=1.0 / SC,
                        scalar2=None, op0=ALU.mult,
                    )
                    nc.sync.dma_start(out=CR_m[64:128, sl], in_=h32s[:])

                # xrows transposes + h/l/CR rows
                xrT32 = scr.tile([64, ROWS], F32)
                nc.vector.memset(Hr66[64:66, :], 1.0)
                hw = ROWS // 2
                for z in range(2):
                    for i in range(z * (n_rt // 8), (z + 1) * (n_rt // 8)):
                        trans_block(xrT32, xrows, i)
                    sl = slice(z * hw, (z + 1) * hw)
                    nc.vector.tensor_copy(out=Hr66[0:64, sl], in_=xrT32[:, sl])
                    resr = halfp.tile([64, hw], F32, tag="resr")
                    nc.vector.tensor_sub(
                        out=resr[:], in0=xrT32[:, sl], in1=Hr66[0:64, sl]
                    )
                    nc.scalar.activation(
                        out=CR_r[0:64, sl], in_=xrT32[:, sl], func=AF.Copy,
                        scale=1.0 / SC,
                    )
                    l32s = halfp.tile([64, hw], F16, tag="l32s")
                    nc.scalar.activation(
                        out=l32s[:], in_=resr[:], func=AF.Copy, scale=SC
                    )
                    nc.sync.dma_start(out=CR_r[64:128, sl], in_=l32s[:])

            # gathers read A_dram; Tile does not track DRAM deps
            tc.strict_bb_all_engine_barrier()

            # ---------- main loop (software-pipelined tail) ----------
            with (
                tc.tile_pool(name="distp", bufs=2) as distp,
                tc.tile_pool(name="work", bufs=3) as work,
                tc.tile_pool(name="psD", bufs=3, space="PSUM") as psD,
            ):
                def emit_tail(t, gath):
                    # A-max over the 8 gathered rows: pairwise folds on gpsimd
                    # (keeps the vector engine free for max8/find_index8)
                    f1 = work.tile([128, 4, U], F16, tag="f1")
                    nc.vector.tensor_tensor(
                        out=f1[:], in0=gath[:, 0:4, :], in1=gath[:, 4:8, :],
                        op=ALU.max,
                    )
                    f2 = work.tile([128, 2, U], F16, tag="f2")
                    nc.vector.tensor_tensor(
                        out=f2[:], in0=f1[:, 0:2, :], in1=f1[:, 2:4, :],
                        op=ALU.max,
                    )
                    pb = psS.tile([128, U], F32, tag="small")
                    nc.tensor.matmul(
                        out=pb[:], lhsT=Hr66[0:65, ts(t, 128)], rhs=rw_h[:, :],
                        start=True, stop=True,
                    )
                    pbs = work.tile([128, U], F32, tag="pbs")
                    nc.scalar.copy(out=pbs[:], in_=pb[:])
                    amax = work.tile([128, U], F32, tag="amax")
                    nc.vector.tensor_tensor(
                        out=amax[:], in0=f2[:, 0, :], in1=f2[:, 1, :], op=ALU.max,
                    )
                    acc = work.tile([128, U], F32, tag="acc")
                    nc.gpsimd.tensor_tensor(
                        out=acc[:], in0=amax[:], in1=pbs[:], op=ALU.add,
                    )
                    # LeakyReLU(0.2): x -> 0.6*x + 0.4*|x| (Pool has no max op)
                    t04 = work.tile([128, U], F32, tag="t04")
                    nc.scalar.activation(
                        out=t04[:], in_=acc[:], func=AF.Abs, scale=0.4
                    )
                    a06 = work.tile([128, U], F32, tag="a06")
                    nc.scalar.activation(
                        out=a06[:], in_=acc[:], func=AF.Copy, scale=0.6
                    )
                    ot = work.tile([128, U], F32, tag="ot")
                    nc.gpsimd.tensor_tensor(
                        out=ot[:], in0=a06[:], in1=t04[:], op=ALU.add,
                    )
                    nc.sync.dma_start(out=out[ts(t, 128), :], in_=ot[:])

                prev = None
                for t in range(n_rt):
                    dist = distp.tile([128, N], F32, tag="dist")
                    for j in range(n_dk):
                        pd = psD.tile([128, 1024], F32, tag="pd")
                        for h in range(2):
                            ck = 2 * j + h
                            nc.tensor.matmul(
                                out=pd[:, ts(h, 512)], lhsT=Hr66[:, ts(t, 128)],
                                rhs=H66[:, ts(ck, 512)], start=True, stop=False,
                            )
                            nc.tensor.matmul(
                                out=pd[:, ts(h, 512)], lhsT=CR_r[:, ts(t, 128)],
                                rhs=CR_m[:, ts(ck, 512)], start=False, stop=True,
                            )
                        nc.scalar.copy(out=dist[:, ts(j, 1024)], in_=pd[:])
                    vals = work.tile([128, K], F32, tag="vals")
                    nc.vector.max(out=vals[:], in_=dist[:])
                    idx = work.tile([128, K], U32, tag="idx")
                    nc.vector.max_index(out=idx[:], in_max=vals[:], in_values=dist[:])
                    gath = work.tile([128, K, U], F16, tag="gath")
                    for k2 in range(K):
                        nc.gpsimd.indirect_dma_start(
                            out=gath[:, k2, :], out_offset=None, in_=A_dram[:],
                            in_offset=bass.IndirectOffsetOnAxis(
                                ap=idx[:, k2:k2 + 1], axis=0
                            ),
                        )
                    if prev is not None:
                        emit_tail(*prev)
                    prev = (t, gath)
                emit_tail(*prev)

    nc.finalize()
    return nc


_NC_CACHE = {}


def _get_nc(N, ROWS, C, U):
    key = (N, ROWS, C, U)
    if key not in _NC_CACHE:
        _NC_CACHE[key] = build_nc(N=N, ROWS=ROWS, C=C, U=U)
    return _NC_CACHE[key]


def kernel(inputs, W, bn_gamma, bn_beta, bn_mean, bn_var, _trace=False):
    """Full-problem entry: takes unsharded inputs, returns (B, N, U) float32."""
    inputs = np.ascontiguousarray(np.asarray(inputs, dtype=np.float32))
    W = np.ascontiguousarray(np.asarray(W, dtype=np.float32))
    bn_gamma = np.asarray(bn_gamma, dtype=np.float32)
    bn_beta = np.asarray(bn_beta, dtype=np.float32)
    bn_mean = np.asarray(bn_mean, dtype=np.float32)
    bn_var = np.asarray(bn_var, dtype=np.float32)

    B, N, C = inputs.shape
    U = W.shape[0]
    assert B * 2 == N_CORES
    ROWS = N // 2

    nc = _get_nc(N, ROWS, C, U)

    in_maps = []
    for c in range(N_CORES):
        b, half = c // 2, c % 2
        in_maps.append({
            "xin": inputs[b],
            "xrows": inputs[b, half * ROWS:(half + 1) * ROWS],
            "W": W,
            "bn_gamma": bn_gamma,
            "bn_beta": bn_beta,
            "bn_mean": bn_mean,
            "bn_var": bn_var,
        })

    res = run_bass_kernel_spmd(nc, in_maps, list(range(N_CORES)), trace=_trace)

    outp = np.empty((B, N, U), dtype=np.float32)
    for c in range(N_CORES):
        b, half = c // 2, c % 2
        outp[b, half * ROWS:(half + 1) * ROWS] = res.results[c]["out"]
    if _trace:
        return outp, res
    return outp
